# revision 23
# baseline (speedup 1.0000x reference)
"""Trainium2 Bass kernel for nn_LinearAttention (RoPE(Q) @ RoPE(Q)^T @ V).

Key algebraic insight: there is no softmax, so
    out = (QR @ QR^T) @ V  ==  QR @ (QR^T @ V)
which replaces the [T,T] score matrix with a [d,d] (64x64) intermediate:
~32x fewer FLOPs. Sharding: 16 heads / 8 cores = 2 heads per core, no
cross-core communication.

Layout: the t-axis is permuted into 16 chunks (t = p*16 + c, p = SBUF
partition). Valid because the contraction sums over all t and the second
matmul is row-local in t; the host packs/unpacks with the same
permutation. The two heads ride in the two 64-partition "lanes" of the
128x128 PE array (head h occupies d-rows/columns 64h:64h+64):

  1. RoPE on Q (DVE + GpSimd share the elementwise work; Q arrives
     pre-split into rotate-half halves so every op is 2D-contiguous).
  2. S2 = sum_c [qr_c(h0)|qr_c(h1)]^T @ [v_c(h0)|v_c(h1)]
     (16 accumulating matmuls N=128; diagonal 64x64 blocks are S_h).
  3. QRT_c = PE-transpose of [qr_c(h0)|qr_c(h1)]  -> both heads' lanes.
  4. outT blocks = blockdiag(S_h0,S_h1)^T @ QRT (4 matmuls N=512; the
     zero off-diagonal blocks kill the cross-head terms).
  5. Four DMAs stream outT out as blocks complete; the host undoes the
     transpose during unsharding.

Perf notes baked in: matmul operands are float32r end-to-end (fp32
streams the moving operand at 2 cycles/column, fp32r at 1); a burst of
dependency-free garbage transposes keeps the PE busy from the preamble
on, so the HAM clock-gate reaches 2.4 GHz before the real matmul
stream; all elementwise ops use fully contiguous 2D access patterns
(multi-dim strided APs hit a DVE slow path ~3x); the Tile kernel-tail
drain+barrier is replaced with a slim per-engine-drain + sem-only
barrier (the default EVSEM butterfly costs ~8 us).

The compiler build allows only ONE sync-wait per engine instruction and
Tile's wait elision is per-engine, so: input DMAs land in SBUF-native
layout (host pre-packs), tiny per-engine "absorber" ops observe each DMA
semaphore once, and cross-engine produced tiles are grouped per consumer
engine. A post-pass splits any remaining multi-wait instruction into
single-wait NoOps.
"""

from contextlib import ExitStack

import numpy as np

import concourse.bass as bass
import concourse.mybir as mybir
import concourse.tile as tile
from concourse.bass_utils import run_bass_kernel_spmd
from concourse.vector_clock import ScopedClock

H, T, D = 16, 2048, 64
N_CORES = 8
HPC = H // N_CORES  # heads per core
P = 128
NT = T // P  # 16 t-chunks per head
HD = D // 2
NTAB = 2 * NT * HPC * HD + P  # cosE | sinE ([2, HPC, 8, HD] each) | idt
F32 = mybir.dt.float32
F32R = mybir.dt.float32r
BF16 = mybir.dt.bfloat16
N_WARM = 22  # dep-free garbage transposes to spin HAM up to 2.4 GHz early


def _rope_tables():
    inv_freq = 1.0 / (10000.0 ** (np.arange(0, D, 2, dtype=np.float32) / D))
    t = np.arange(T, dtype=np.float32)
    freqs = np.outer(t, inv_freq).astype(np.float32)  # [T, D/2]
    return np.cos(freqs).astype(np.float32), np.sin(freqs).astype(np.float32)


class _SlimTileContext(tile.TileContext):
    """TileContext whose kernel tail uses per-engine drains + a
    sequencer-level (sem-only) barrier instead of the full EVSEM
    butterfly. Semantics kept: SP's drain still waits on every live
    semaphore's final value (split into single-wait NoOps later), each
    engine's pipeline is drained before the semaphore range-clear, and a
    final sem-only barrier orders the clear before the NEFF ends."""

    def _drain_and_barrier(self, tick_clock, wait_clock):
        nc = self.nc
        drain_inst = nc.sync.drain()
        wait_clock.add_sem_waits(
            drain_inst.ins, ScopedClock({None: tick_clock.global_clock})
        )
        for eng in nc.engines.values():
            if eng.engine != mybir.EngineType.SP:
                eng.drain(fusable=False)
        nc.all_engine_barrier(sem_only=True)
        popped = nc._tile_sem_poison_stack.pop()
        assert popped is self._sem_poison
        nc.clear_and_free_semaphores(list(self.sems.allocated().values()))
        nc.all_engine_barrier(sem_only=True)


def _build_nc():
    nc = bass.Bass()
    TAB = nc.declare_dram_parameter("TAB", [P, NTAB], BF16, isOutput=False)
    # q pre-split into rotate-half halves: [head, half, chunk, k]
    QA = nc.declare_dram_parameter("QA", [P, HPC * 2 * 8 * HD], BF16, isOutput=False)
    QB = nc.declare_dram_parameter("QB", [P, HPC * 2 * 8 * HD], BF16, isOutput=False)
    VA = nc.declare_dram_parameter("VA", [P, 8 * HPC * D], BF16, isOutput=False)
    VB = nc.declare_dram_parameter("VB", [P, 8 * HPC * D], BF16, isOutput=False)
    OUT = nc.declare_dram_parameter("OUT", [P, T], BF16, isOutput=True)

    with _SlimTileContext(nc) as tc, ExitStack() as ctx:
        singles = ctx.enter_context(tc.tile_pool(name="singles", bufs=1))
        ps_s = ctx.enter_context(tc.tile_pool(name="ps_s", bufs=1, space="PSUM"))
        ps_tp = ctx.enter_context(tc.tile_pool(name="ps_tp", bufs=3, space="PSUM"))
        ps_o = ctx.enter_context(tc.tile_pool(name="ps_o", bufs=2, space="PSUM"))

        # Garbage-input PE warm-up: no data dependencies at all, so these
        # start right after the engine preamble and keep the PE busy
        # while the input DMAs land (HAM reaches 8/8 before real work).
        spam_src = singles.tile([P, P], F32)
        nc.gpsimd.memset(spam_src[:, 0:2], 0.0)
        for _ in range(N_WARM):
            warm = ps_tp.tile([P, P], F32, tag="tp")
            nc.tensor.transpose(warm, spam_src, spam_src)

        tab_sb = singles.tile([P, NTAB], BF16)
        # q layout: [range, half, head, chunk-in-range, k]
        q_sb = singles.tile([P, 2, 2, HPC, 8, HD], BF16)
        v_sb = singles.tile([P, NT, HPC, D], BF16)
        # Two HWDGE rings in parallel, ordered so RoPE's inputs land
        # first: SP streams QA then QB; ACT streams TAB then VA, VB.
        nc.sync.dma_start(
            out=q_sb[:, 0],
            in_=QA[:].rearrange("p (x h c k) -> p x h c k", x=2, h=HPC, c=8),
        )
        nc.scalar.dma_start(out=tab_sb, in_=TAB[:])
        nc.sync.dma_start(
            out=q_sb[:, 1],
            in_=QB[:].rearrange("p (x h c k) -> p x h c k", x=2, h=HPC, c=8),
        )
        nc.scalar.dma_start(
            out=v_sb[:, 0:8],
            in_=VA[:].rearrange("p (c h d) -> p c h d", c=8, h=HPC),
        )
        nc.scalar.dma_start(
            out=v_sb[:, 8:16],
            in_=VB[:].rearrange("p (c h d) -> p c h d", c=8, h=HPC),
        )

        idt = tab_sb[:, 2 * NT * HPC * HD :]

        qr_r = singles.tile([P, NT, HPC, 2, HD], BF16)
        qrtmp = singles.tile([P, 2, HPC * 8 * HD], BF16)
        tmp1 = singles.tile([P, HPC * 8 * HD], BF16)
        tmp2 = singles.tile([P, HPC * 8 * HD], BF16)
        qrt_sb = singles.tile([P, NT * P], BF16)
        s2d = singles.tile([P, P], BF16)
        outT_sb = singles.tile([P, T], BF16)
        scratch = singles.tile([P, 8], F32)

        # Absorbers + early table work (DVE and GpSimd observe the TAB
        # semaphore; the off-diagonal zeros of the phase-3 operand only
        # need the identity slab, so they run while waiting for Q/V).
        idt_r = singles.tile([P, P], BF16)
        nc.vector.tensor_copy(out=idt_r, in_=idt)
        nc.vector.tensor_scalar_mul(s2d[:D, D:], idt[:D, :D], 0.0)
        nc.vector.tensor_scalar_mul(s2d[D:, :D], idt[:D, :D], 0.0)

        s2_ps = ps_s.tile([P, P], F32)

        nexp = HPC * 8 * HD
        for half in range(2):
            r0 = half * 8
            cs = slice(r0, r0 + 8)
            cosr = tab_sb[:, half * nexp : (half + 1) * nexp]
            sinr = tab_sb[:, (2 + half) * nexp : (3 + half) * nexp]

            # RoPE over a whole chunk-range, both heads, per rotate-half
            # half: 6 ops of [128, 512], fully contiguous except the two
            # final writes (which scatter into the chunk-major qr tile in
            # matching (h, c, k) iteration order).
            #   qr_lo = q_lo*cos - q_hi*sin ; qr_hi = q_hi*cos + q_lo*sin
            qlo = q_sb[:, half, 0].rearrange("p h c k -> p (h c k)")
            qhi = q_sb[:, half, 1].rearrange("p h c k -> p (h c k)")
            nc.vector.tensor_mul(tmp1, qhi, sinr)
            nc.vector.tensor_mul(tmp2, qlo, sinr)
            nc.vector.tensor_mul(qrtmp[:, 0], qlo, cosr)
            nc.vector.tensor_mul(qrtmp[:, 1], qhi, cosr)
            qr_lo = qr_r[:, cs, :, 0, :].rearrange("p c h k -> p h c k")
            qr_hi = qr_r[:, cs, :, 1, :].rearrange("p c h k -> p h c k")
            shp = dict(h=HPC, c=8)
            nc.vector.tensor_sub(
                qr_lo, qrtmp[:, 0].rearrange("p (h c k) -> p h c k", **shp),
                tmp1.rearrange("p (h c k) -> p h c k", **shp),
            )
            nc.vector.tensor_add(
                qr_hi, qrtmp[:, 1].rearrange("p (h c k) -> p h c k", **shp),
                tmp2.rearrange("p (h c k) -> p h c k", **shp),
            )

            # PE observes this half's v-DMA semaphore once (result unused).
            warm2 = ps_tp.tile([P, P], BF16, tag="tp")
            nc.tensor.transpose(
                warm2, v_sb[:, r0].rearrange("p h d -> p (h d)"), idt_r
            )
            if half == 1:
                # Filler keeps the PE's activity window busy while DVE
                # finishes this range's RoPE, so HAM stays at 8/8.
                for _ in range(8):
                    warm_f = ps_tp.tile([P, P], F32, tag="tp")
                    nc.tensor.transpose(warm_f, spam_src, spam_src)

            for c in range(r0, r0 + 8):
                # lhsT free order (h, half, k) = (h, d): the head lanes.
                qr2 = qr_r[:, c].rearrange("p h x k -> p (h x k)")
                v2 = v_sb[:, c].rearrange("p h d -> p (h d)")
                nc.tensor.matmul(
                    s2_ps, lhsT=qr2, rhs=v2, start=(c == 0), stop=(c == NT - 1)
                )
                # Transpose as a REGULAR matmul with the identity as the
                # moving operand (qr_c^T @ I): the moving-operand slot
                # requires a single free dimension, which qr2 (multi-dim
                # lhsT AP) cannot satisfy in transpose mode.
                tp = ps_tp.tile([P, P], F32, tag="tp")
                nc.tensor.matmul(tp, lhsT=qr2, rhs=idt_r, start=True, stop=True)
                # chunks 0-7 copy on ACT, 8-15 on DVE: splits the copy
                # load; phase 3's single DVE wait covers the DVE-copied
                # chunks transitively (s2d is copied later on DVE), and
                # the warm3 absorber covers the ACT-copied ones.
                if c < 8:
                    nc.scalar.copy(out=qrt_sb[:, c * P : (c + 1) * P], in_=tp)
                else:
                    nc.vector.tensor_copy(out=qrt_sb[:, c * P : (c + 1) * P], in_=tp)

        # Diagonal S_h blocks -> block-diagonal phase-3 operand.
        nc.vector.tensor_copy(out=s2d[:D, :D], in_=s2_ps[:D, :D])
        nc.vector.tensor_copy(out=s2d[D:, D:], in_=s2_ps[D:, D:])

        # PE observes the ACT semaphore once (after the last qrt copy).
        warm3 = ps_s.tile([8, P], BF16, tag="w3")
        nc.tensor.transpose(warm3, qrt_sb[:, 8 * P - 8 : 8 * P], idt_r)

        # outT blocks: blockdiag(S)^T @ QRT serves both heads at once.
        for i in range(4):
            o_ps = ps_o.tile([P, 512], F32, tag="o")
            blk = slice(i * 512, (i + 1) * 512)
            nc.tensor.matmul(
                o_ps, lhsT=s2d, rhs=qrt_sb[:, blk], start=True, stop=True
            )
            if i < 3:
                nc.scalar.copy(out=outT_sb[:, blk], in_=o_ps)
            else:
                nc.vector.tensor_copy(out=outT_sb[:, blk], in_=o_ps)
            nc.sync.dma_start(out=OUT[:, blk], in_=outT_sb[:, blk])

    _split_multi_waits(nc)
    return nc


def _split_multi_waits(nc):
    """This compiler build rejects instructions carrying more than one
    sync-wait command. Tile's kernel-tail drain aggregates one wait per
    live semaphore, so split the extras into single-wait NoOps placed
    immediately before it on the same engine (sequential execution on the
    engine's queue preserves the barrier semantics)."""
    n = 0
    for f in nc.m.functions:
        for blk in f.blocks:
            new_insts = []
            for inst in blk.instructions:
                si = inst.sync_info
                waits = list(si.on_wait) if si else []
                if len(waits) > 1:
                    for w in waits[:-1]:
                        nop = mybir.InstNoOp(name=f"W-split-{n}", ins=[], outs=[])
                        n += 1
                        nop.engine = inst.engine
                        nop.sync_info = mybir.SyncInfo(on_wait=[w], on_update=[])
                        new_insts.append(nop)
                    inst.sync_info = mybir.SyncInfo(
                        on_wait=[waits[-1]], on_update=list(si.on_update)
                    )
                new_insts.append(inst)
            blk.instructions = new_insts


_NC_CACHE = None


def _get_nc():
    global _NC_CACHE
    if _NC_CACHE is None:
        _NC_CACHE = _build_nc()
    return _NC_CACHE


def _pack_inputs(Qs, Vs, cos32, sin32, idt):
    import ml_dtypes

    bf16 = ml_dtypes.bfloat16

    # [T, X] -> [P, NT, X] with t = p*NT + c
    def r(x):
        return x.reshape(P, NT, -1)

    # cosE[p, range, h, c, k] = cos32[t = p*16 + range*8 + c, k]
    ce = r(cos32).reshape(P, 2, 8, HD)  # [p, range, c, k]
    se = r(sin32).reshape(P, 2, 8, HD)
    cosE = np.repeat(ce[:, :, None, :, :], HPC, axis=2)  # [p, range, h, c, k]
    sinE = np.repeat(se[:, :, None, :, :], HPC, axis=2)
    tab = np.concatenate(
        [cosE.reshape(P, -1), sinE.reshape(P, -1), idt], axis=1
    ).astype(bf16)
    tab = np.ascontiguousarray(tab)

    in_maps = []
    for core in range(N_CORES):
        h0 = core * HPC
        # q[p, range, half, h, c, k], v[p, c, h, d]
        q = np.empty((P, 2, 2, HPC, 8, HD), np.float32)
        v = np.empty((P, NT, HPC, D), np.float32)
        for h in range(HPC):
            qh = r(Qs[h0 + h]).reshape(P, 2, 8, D)  # [p, range, c, d]
            q[:, :, 0, h] = qh[:, :, :, :HD]
            q[:, :, 1, h] = qh[:, :, :, HD:]
            v[:, :, h] = r(Vs[h0 + h])
        in_maps.append(
            {
                "TAB": tab,
                "QA": np.ascontiguousarray(q[:, 0].reshape(P, -1).astype(bf16)),
                "QB": np.ascontiguousarray(q[:, 1].reshape(P, -1).astype(bf16)),
                "VA": np.ascontiguousarray(v[:, 0:8].reshape(P, -1).astype(bf16)),
                "VB": np.ascontiguousarray(v[:, 8:16].reshape(P, -1).astype(bf16)),
            }
        )
    return in_maps


def _unpack_out(o):
    # o: [P, T] = outT; rows h*64+j, cols c-major: col = c*128 + f, t = f*16+c
    a = o.reshape(HPC, D, NT, P)  # [h, j, c, f]
    return a.transpose(0, 3, 2, 1).reshape(HPC, T, D)  # [h, t=f*16+c, j]


def run_inner(Q, K, V, trace=False):
    del K  # the module sets KR = QR; K is unused
    Qs = np.asarray(Q, dtype=np.float32)[0]  # [H, T, D]
    Vs = np.asarray(V, dtype=np.float32)[0]
    cos32, sin32 = _rope_tables()
    idt = np.eye(P, dtype=np.float32)
    nc = _get_nc()
    in_maps = _pack_inputs(Qs, Vs, cos32, sin32, idt)
    res = run_bass_kernel_spmd(nc, in_maps, list(range(N_CORES)), trace=trace)
    outs = [_unpack_out(np.asarray(res.results[i]["OUT"])) for i in range(N_CORES)]
    out = np.concatenate(outs, axis=0)[None]  # [1, H, T, D]
    return out.astype(np.float32), res


def kernel(Q, K, V):
    out, _ = run_inner(Q, K, V, trace=False)
    return out


# revision 24
# speedup vs baseline: 1.0847x; 1.0847x over previous
"""Trainium2 Bass kernel for nn_LinearAttention (RoPE(Q) @ RoPE(Q)^T @ V).

Key algebraic insight: there is no softmax, so
    out = (QR @ QR^T) @ V  ==  QR @ (QR^T @ V)
which replaces the [T,T] score matrix with a [d,d] (64x64) intermediate:
~32x fewer FLOPs. Sharding: 16 heads / 8 cores = 2 heads per core, no
cross-core communication.

Layout: the t-axis is permuted into 16 chunks (t = p*16 + c, p = SBUF
partition). Valid because the contraction sums over all t and the second
matmul is row-local in t; the host packs/unpacks with the same
permutation. The two heads ride in the two 64-partition "lanes" of the
128x128 PE array (head h occupies d-rows/columns 64h:64h+64):

  1. RoPE on Q (DVE + GpSimd share the elementwise work; Q arrives
     pre-split into rotate-half halves so every op is 2D-contiguous).
  2. S2 = sum_c [qr_c(h0)|qr_c(h1)]^T @ [v_c(h0)|v_c(h1)]
     (16 accumulating matmuls N=128; diagonal 64x64 blocks are S_h).
  3. QRT_c = PE-transpose of [qr_c(h0)|qr_c(h1)]  -> both heads' lanes.
  4. outT blocks = blockdiag(S_h0,S_h1)^T @ QRT (4 matmuls N=512; the
     zero off-diagonal blocks kill the cross-head terms).
  5. Four DMAs stream outT out as blocks complete; the host undoes the
     transpose during unsharding.

Perf notes baked in: matmul operands are float32r end-to-end (fp32
streams the moving operand at 2 cycles/column, fp32r at 1); a burst of
dependency-free garbage transposes keeps the PE busy from the preamble
on, so the HAM clock-gate reaches 2.4 GHz before the real matmul
stream; all elementwise ops use fully contiguous 2D access patterns
(multi-dim strided APs hit a DVE slow path ~3x); the Tile kernel-tail
drain+barrier is replaced with a slim per-engine-drain + sem-only
barrier (the default EVSEM butterfly costs ~8 us).

The compiler build allows only ONE sync-wait per engine instruction and
Tile's wait elision is per-engine, so: input DMAs land in SBUF-native
layout (host pre-packs), tiny per-engine "absorber" ops observe each DMA
semaphore once, and cross-engine produced tiles are grouped per consumer
engine. A post-pass splits any remaining multi-wait instruction into
single-wait NoOps.
"""

from contextlib import ExitStack

import numpy as np

import concourse.bass as bass
import concourse.mybir as mybir
import concourse.tile as tile
from concourse.bass_utils import run_bass_kernel_spmd
from concourse.vector_clock import ScopedClock

H, T, D = 16, 2048, 64
N_CORES = 8
HPC = H // N_CORES  # heads per core
P = 128
NT = T // P  # 16 t-chunks per head
HD = D // 2
NTAB = 2 * NT * HPC * HD + P  # cosE | sinE ([2, HPC, 8, HD] each) | idt
F32 = mybir.dt.float32
F32R = mybir.dt.float32r
BF16 = mybir.dt.bfloat16
N_WARM = 22  # dep-free garbage transposes to spin HAM up to 2.4 GHz early


def _rope_tables():
    inv_freq = 1.0 / (10000.0 ** (np.arange(0, D, 2, dtype=np.float32) / D))
    t = np.arange(T, dtype=np.float32)
    freqs = np.outer(t, inv_freq).astype(np.float32)  # [T, D/2]
    return np.cos(freqs).astype(np.float32), np.sin(freqs).astype(np.float32)


class _SlimTileContext(tile.TileContext):
    """TileContext whose kernel tail uses per-engine drains + a
    sequencer-level (sem-only) barrier instead of the full EVSEM
    butterfly. Semantics kept: SP's drain still waits on every live
    semaphore's final value (split into single-wait NoOps later), each
    engine's pipeline is drained before the semaphore range-clear, and a
    final sem-only barrier orders the clear before the NEFF ends."""

    def _drain_and_barrier(self, tick_clock, wait_clock):
        nc = self.nc
        drain_inst = nc.sync.drain()
        wait_clock.add_sem_waits(
            drain_inst.ins, ScopedClock({None: tick_clock.global_clock})
        )
        for eng in nc.engines.values():
            if eng.engine != mybir.EngineType.SP:
                eng.drain(fusable=False)
        nc.all_engine_barrier(sem_only=True)
        popped = nc._tile_sem_poison_stack.pop()
        assert popped is self._sem_poison
        nc.clear_and_free_semaphores(list(self.sems.allocated().values()))
        nc.all_engine_barrier(sem_only=True)


def _build_nc():
    nc = bass.Bass()
    TAB = nc.declare_dram_parameter("TAB", [P, NTAB], BF16, isOutput=False)
    # q pre-split into rotate-half halves: [head, half, chunk, k]
    QA = nc.declare_dram_parameter("QA", [P, HPC * 2 * 8 * HD], BF16, isOutput=False)
    QB = nc.declare_dram_parameter("QB", [P, HPC * 2 * 8 * HD], BF16, isOutput=False)
    VA = nc.declare_dram_parameter("VA", [P, 8 * HPC * D], BF16, isOutput=False)
    VB = nc.declare_dram_parameter("VB", [P, 8 * HPC * D], BF16, isOutput=False)
    OUT = nc.declare_dram_parameter("OUT", [P, T], BF16, isOutput=True)

    with _SlimTileContext(nc) as tc, ExitStack() as ctx:
        singles = ctx.enter_context(tc.tile_pool(name="singles", bufs=1))
        ps_s = ctx.enter_context(tc.tile_pool(name="ps_s", bufs=1, space="PSUM"))
        ps_tp = ctx.enter_context(tc.tile_pool(name="ps_tp", bufs=3, space="PSUM"))
        ps_o = ctx.enter_context(tc.tile_pool(name="ps_o", bufs=2, space="PSUM"))

        # Garbage-input PE warm-up: no data dependencies at all, so these
        # start right after the engine preamble and keep the PE busy
        # while the input DMAs land (HAM reaches 8/8 before real work).
        spam_src = singles.tile([P, P], F32)
        nc.gpsimd.memset(spam_src[:, 0:2], 0.0)
        for _ in range(N_WARM):
            warm = ps_tp.tile([P, P], F32, tag="tp")
            nc.tensor.transpose(warm, spam_src, spam_src)

        tab_sb = singles.tile([P, NTAB], BF16)
        # q layout: [range, half, head, chunk-in-range, k]
        q_sb = singles.tile([P, 2, 2, HPC, 8, HD], BF16)
        v_sb = singles.tile([P, NT, HPC, D], BF16)
        # Two HWDGE rings in parallel, ordered so RoPE's inputs land
        # first: SP streams QA then QB; ACT streams TAB then VA, VB.
        nc.sync.dma_start(
            out=q_sb[:, 0],
            in_=QA[:].rearrange("p (x h c k) -> p x h c k", x=2, h=HPC, c=8),
        )
        nc.scalar.dma_start(out=tab_sb, in_=TAB[:])
        nc.sync.dma_start(
            out=q_sb[:, 1],
            in_=QB[:].rearrange("p (x h c k) -> p x h c k", x=2, h=HPC, c=8),
        )
        nc.scalar.dma_start(
            out=v_sb[:, 0:8],
            in_=VA[:].rearrange("p (c h d) -> p c h d", c=8, h=HPC),
        )
        nc.scalar.dma_start(
            out=v_sb[:, 8:16],
            in_=VB[:].rearrange("p (c h d) -> p c h d", c=8, h=HPC),
        )

        idt = tab_sb[:, 2 * NT * HPC * HD :]

        qr_r = singles.tile([P, NT, HPC, 2, HD], BF16)
        qrtmp = singles.tile([P, 2, HPC * 8 * HD], BF16)
        tmp1 = singles.tile([P, HPC * 8 * HD], BF16)
        tmp2 = singles.tile([P, HPC * 8 * HD], BF16)
        qrt_sb = singles.tile([P, NT * P], BF16)
        s2d = singles.tile([P, P], BF16)
        outT_sb = singles.tile([P, T], BF16)
        scratch = singles.tile([P, 8], F32)

        # Absorbers + early table work (DVE and GpSimd observe the TAB
        # semaphore; the off-diagonal zeros of the phase-3 operand only
        # need the identity slab, so they run while waiting for Q/V).
        idt_r = singles.tile([P, P], BF16)
        nc.vector.tensor_copy(out=idt_r, in_=idt)
        nc.vector.tensor_scalar_mul(s2d[:D, D:], idt[:D, :D], 0.0)
        nc.vector.tensor_scalar_mul(s2d[D:, :D], idt[:D, :D], 0.0)

        s2_ps = ps_s.tile([P, P], F32)

        nexp = HPC * 8 * HD
        for half in range(2):
            r0 = half * 8
            cs = slice(r0, r0 + 8)
            cosr = tab_sb[:, half * nexp : (half + 1) * nexp]
            sinr = tab_sb[:, (2 + half) * nexp : (3 + half) * nexp]

            # RoPE over a whole chunk-range, both heads, per rotate-half
            # half: 6 ops of [128, 512], fully contiguous except the two
            # final writes (which scatter into the chunk-major qr tile in
            # matching (h, c, k) iteration order).
            #   qr_lo = q_lo*cos - q_hi*sin ; qr_hi = q_hi*cos + q_lo*sin
            qlo = q_sb[:, half, 0].rearrange("p h c k -> p (h c k)")
            qhi = q_sb[:, half, 1].rearrange("p h c k -> p (h c k)")
            nc.vector.tensor_mul(tmp1, qhi, sinr)
            nc.vector.tensor_mul(tmp2, qlo, sinr)
            nc.vector.tensor_mul(qrtmp[:, 0], qlo, cosr)
            nc.vector.tensor_mul(qrtmp[:, 1], qhi, cosr)
            qr_lo = qr_r[:, cs, :, 0, :].rearrange("p c h k -> p h c k")
            qr_hi = qr_r[:, cs, :, 1, :].rearrange("p c h k -> p h c k")
            shp = dict(h=HPC, c=8)
            nc.vector.tensor_sub(
                qr_lo, qrtmp[:, 0].rearrange("p (h c k) -> p h c k", **shp),
                tmp1.rearrange("p (h c k) -> p h c k", **shp),
            )
            nc.vector.tensor_add(
                qr_hi, qrtmp[:, 1].rearrange("p (h c k) -> p h c k", **shp),
                tmp2.rearrange("p (h c k) -> p h c k", **shp),
            )

            # PE observes this half's v-DMA semaphore once (result unused).
            warm2 = ps_tp.tile([P, P], BF16, tag="tp")
            nc.tensor.transpose(
                warm2, v_sb[:, r0].rearrange("p h d -> p (h d)"), idt_r
            )
            if half == 0:
                # Filler bridges the PE idle window between the warm-up
                # burst and the first real matmuls (RoPE-A still running
                # on DVE), so HAM's MID window never sees ~2 us of idle.
                for _ in range(6):
                    warm_f = ps_tp.tile([P, P], F32, tag="tp")
                    nc.tensor.transpose(warm_f, spam_src, spam_src)

            for c in range(r0, r0 + 8):
                # lhsT free order (h, half, k) = (h, d): the head lanes.
                qr2 = qr_r[:, c].rearrange("p h x k -> p (h x k)")
                v2 = v_sb[:, c].rearrange("p h d -> p (h d)")
                nc.tensor.matmul(
                    s2_ps, lhsT=qr2, rhs=v2, start=(c == 0), stop=(c == NT - 1)
                )
                # Transpose as a REGULAR matmul with the identity as the
                # moving operand (qr_c^T @ I): the moving-operand slot
                # requires a single free dimension, which qr2 (multi-dim
                # lhsT AP) cannot satisfy in transpose mode.
                tp = ps_tp.tile([P, P], F32, tag="tp")
                nc.tensor.matmul(tp, lhsT=qr2, rhs=idt_r, start=True, stop=True)
                # chunks 0-7 copy on ACT, 8-15 on DVE: splits the copy
                # load; phase 3's single DVE wait covers the DVE-copied
                # chunks transitively (s2d is copied later on DVE), and
                # the warm3 absorber covers the ACT-copied ones.
                if c < 8:
                    nc.scalar.copy(out=qrt_sb[:, c * P : (c + 1) * P], in_=tp)
                else:
                    nc.vector.tensor_copy(out=qrt_sb[:, c * P : (c + 1) * P], in_=tp)

        # Diagonal S_h blocks -> block-diagonal phase-3 operand.
        nc.vector.tensor_copy(out=s2d[:D, :D], in_=s2_ps[:D, :D])
        nc.vector.tensor_copy(out=s2d[D:, D:], in_=s2_ps[D:, D:])

        # PE observes the ACT semaphore once (after the last qrt copy).
        warm3 = ps_s.tile([8, P], BF16, tag="w3")
        nc.tensor.transpose(warm3, qrt_sb[:, 8 * P - 8 : 8 * P], idt_r)

        # outT blocks: blockdiag(S)^T @ QRT serves both heads at once.
        for i in range(4):
            o_ps = ps_o.tile([P, 512], F32, tag="o")
            blk = slice(i * 512, (i + 1) * 512)
            nc.tensor.matmul(
                o_ps, lhsT=s2d, rhs=qrt_sb[:, blk], start=True, stop=True
            )
            nc.vector.tensor_copy(out=outT_sb[:, blk], in_=o_ps)
            nc.sync.dma_start(out=OUT[:, blk], in_=outT_sb[:, blk])

    _split_multi_waits(nc)
    return nc


def _split_multi_waits(nc):
    """This compiler build rejects instructions carrying more than one
    sync-wait command. Tile's kernel-tail drain aggregates one wait per
    live semaphore, so split the extras into single-wait NoOps placed
    immediately before it on the same engine (sequential execution on the
    engine's queue preserves the barrier semantics)."""
    n = 0
    for f in nc.m.functions:
        for blk in f.blocks:
            new_insts = []
            for inst in blk.instructions:
                si = inst.sync_info
                waits = list(si.on_wait) if si else []
                if len(waits) > 1:
                    for w in waits[:-1]:
                        nop = mybir.InstNoOp(name=f"W-split-{n}", ins=[], outs=[])
                        n += 1
                        nop.engine = inst.engine
                        nop.sync_info = mybir.SyncInfo(on_wait=[w], on_update=[])
                        new_insts.append(nop)
                    inst.sync_info = mybir.SyncInfo(
                        on_wait=[waits[-1]], on_update=list(si.on_update)
                    )
                new_insts.append(inst)
            blk.instructions = new_insts


_NC_CACHE = None


def _get_nc():
    global _NC_CACHE
    if _NC_CACHE is None:
        _NC_CACHE = _build_nc()
    return _NC_CACHE


def _pack_inputs(Qs, Vs, cos32, sin32, idt):
    import ml_dtypes

    bf16 = ml_dtypes.bfloat16

    # [T, X] -> [P, NT, X] with t = p*NT + c
    def r(x):
        return x.reshape(P, NT, -1)

    # cosE[p, range, h, c, k] = cos32[t = p*16 + range*8 + c, k]
    ce = r(cos32).reshape(P, 2, 8, HD)  # [p, range, c, k]
    se = r(sin32).reshape(P, 2, 8, HD)
    cosE = np.repeat(ce[:, :, None, :, :], HPC, axis=2)  # [p, range, h, c, k]
    sinE = np.repeat(se[:, :, None, :, :], HPC, axis=2)
    tab = np.concatenate(
        [cosE.reshape(P, -1), sinE.reshape(P, -1), idt], axis=1
    ).astype(bf16)
    tab = np.ascontiguousarray(tab)

    in_maps = []
    for core in range(N_CORES):
        h0 = core * HPC
        # q[p, range, half, h, c, k], v[p, c, h, d]
        q = np.empty((P, 2, 2, HPC, 8, HD), np.float32)
        v = np.empty((P, NT, HPC, D), np.float32)
        for h in range(HPC):
            qh = r(Qs[h0 + h]).reshape(P, 2, 8, D)  # [p, range, c, d]
            q[:, :, 0, h] = qh[:, :, :, :HD]
            q[:, :, 1, h] = qh[:, :, :, HD:]
            v[:, :, h] = r(Vs[h0 + h])
        in_maps.append(
            {
                "TAB": tab,
                "QA": np.ascontiguousarray(q[:, 0].reshape(P, -1).astype(bf16)),
                "QB": np.ascontiguousarray(q[:, 1].reshape(P, -1).astype(bf16)),
                "VA": np.ascontiguousarray(v[:, 0:8].reshape(P, -1).astype(bf16)),
                "VB": np.ascontiguousarray(v[:, 8:16].reshape(P, -1).astype(bf16)),
            }
        )
    return in_maps


def _unpack_out(o):
    # o: [P, T] = outT; rows h*64+j, cols c-major: col = c*128 + f, t = f*16+c
    a = o.reshape(HPC, D, NT, P)  # [h, j, c, f]
    return a.transpose(0, 3, 2, 1).reshape(HPC, T, D)  # [h, t=f*16+c, j]


def run_inner(Q, K, V, trace=False):
    del K  # the module sets KR = QR; K is unused
    Qs = np.asarray(Q, dtype=np.float32)[0]  # [H, T, D]
    Vs = np.asarray(V, dtype=np.float32)[0]
    cos32, sin32 = _rope_tables()
    idt = np.eye(P, dtype=np.float32)
    nc = _get_nc()
    in_maps = _pack_inputs(Qs, Vs, cos32, sin32, idt)
    res = run_bass_kernel_spmd(nc, in_maps, list(range(N_CORES)), trace=trace)
    outs = [_unpack_out(np.asarray(res.results[i]["OUT"])) for i in range(N_CORES)]
    out = np.concatenate(outs, axis=0)[None]  # [1, H, T, D]
    return out.astype(np.float32), res


def kernel(Q, K, V):
    out, _ = run_inner(Q, K, V, trace=False)
    return out
